# revision 1
# baseline (speedup 1.0000x reference)
"""Trainium2 Bass kernel for nn_LogisticRegression (multi-hot + mean-embedding
logistic regression over a 50k vocab).

Math: for each row i with tokens x[i, 0:200]:
    logit[i] = b + (1/200) * sum_j s[x_ij] + sum_{unique tokens t in row} Wv[t]
    y[i] = sigmoid(logit[i])
where s = E @ w_emb (one scalar per vocab entry), Wv = W[0, 300:].

Device strategy (8 NeuronCores, SPMD):
  - batch-shard rows: 128 rows per core.
  - vocab-shard the s computation: each core streams its 6250-row slice of E
    (host-padded to 6272 rows) and reduces s_shard (DVE mult + ACT accum);
    a 25KB AllGather makes s global.
  - each core builds a value table vt in DRAM: 64B per vocab entry packed as
    [s/200, Wv, 14*pad]; a 256B dma_gather element covers 4 entries.
  - tokens are SORTED per row on DVE via Max8 + MatchReplace (25 rounds);
    sorting preserves multiplicity, so the s-part sums all occurrences while
    the exact unique-token mask for Wv is one adjacent not_equal compare.
  - per-token values come from dma_gather (the MoE gather): idx = x_sorted>>2
    (fits int16), then a 1-of-4 lane select on DVE.
"""
import sys

sys.path.insert(0, "/opt/trn_rl_repo")

import numpy as np

import concourse.bass as bass
import concourse.bacc as bacc
import concourse.mybir as mybir
import concourse.tile as tile

N_CORES = 8
P = 128            # rows per core (batch 1024 / 8)
L = 200            # tokens per row
VOCAB = 50000
EMB = 300
VSH = 6250         # vocab shard per core
VSHP = 6272        # padded E shard rows (49 * 128)
TPP = 49           # E rows per partition (6272 / 128)
NG = 7             # E DMA groups (7 tiles each)
VTOT = 50176       # padded vocab entries in vt (128 * 392)
EPP = VTOT // P    # vt entries per partition (392)
VT_ROW = 16        # f32 per vocab entry in vt (64B)
GB = 4             # vocab entries per 256B gather block
NCHUNK = 8         # gather chunks
JC = L // NCHUNK   # token slots per chunk (25)
F32 = mybir.dt.float32
I32 = mybir.dt.int32
I16 = mybir.dt.int16

_CACHE = {}


def build_nc():
    nc = bacc.Bacc("TRN2", target_bir_lowering=False, debug=True)
    x_in = nc.dram_tensor("x_in", [P, L], I32, kind="ExternalInput")
    e_sh = nc.dram_tensor("e_sh", [VSHP, EMB], F32, kind="ExternalInput")
    wemb = nc.dram_tensor("wemb", [1, EMB], F32, kind="ExternalInput")
    wv_all = nc.dram_tensor("wv_all", [P, EPP], F32, kind="ExternalInput")
    bias_in = nc.dram_tensor("bias_in", [1, 1], F32, kind="ExternalInput")
    y_out = nc.dram_tensor("y_out", [P, 1], F32, kind="ExternalOutput")

    s_shard = nc.dram_tensor("s_shard", [1, VSHP], F32)
    s_full = nc.dram_tensor("s_full", [1, VTOT], F32, addr_space="Shared")
    vt = nc.dram_tensor("vt", [P, EPP * VT_ROW], F32)   # 64B per entry

    with tile.TileContext(nc) as tc:
        with (
            tc.tile_pool(name="sb", bufs=1) as pool,
            tc.tile_pool(name="et", bufs=2) as epool,
            tc.tile_pool(name="gat", bufs=2) as gpool,
        ):
            # ---- inputs to SBUF -------------------------------------------
            x_sb = pool.tile([P, L], I32)
            nc.sync.dma_start(out=x_sb[:], in_=x_in[:])
            xf = pool.tile([P, L], F32)
            nc.vector.tensor_copy(out=xf[:], in_=x_sb[:])
            wb = pool.tile([P, EMB], F32)
            nc.sync.dma_start(out=wb[:], in_=wemb[0:1, :].to_broadcast([P, EMB]))
            bb = pool.tile([P, 1], F32)
            nc.sync.dma_start(out=bb[:], in_=bias_in[0:1, :].to_broadcast([P, 1]))

            # ---- vt skeleton: zeros + Wv lanes (independent of s) ---------
            # vt entry v lives at partition v // 392, f32 col 16 * (v % 392).
            vt_sb = pool.tile([P, EPP * VT_ROW], F32)  # 6272 f32/partition
            nc.vector.memset(vt_sb[:], 0.0)
            wv_sb = pool.tile([P, EPP], F32)
            nc.sync.dma_start(out=wv_sb[:], in_=wv_all[:])
            nc.vector.tensor_copy(
                out=vt_sb[:, 1:EPP * VT_ROW:VT_ROW], in_=wv_sb[:]
            )

            # ---- sort each row descending (Max8 + MatchReplace) -----------
            xs = pool.tile([P, L], F32)       # sorted tokens
            work = pool.tile([P, L], F32)
            nc.vector.tensor_copy(out=work[:], in_=xf[:])
            for k in range(L // 8):
                nc.vector.max(out=xs[:, 8 * k:8 * k + 8], in_=work[:])
                nc.vector.match_replace(
                    out=work[:],
                    in_to_replace=xs[:, 8 * k:8 * k + 8],
                    in_values=work[:],
                    imm_value=-1.0,
                )
            # unique-token weight: 1 at the first slot of each equal-run
            wdup = pool.tile([P, L], F32)
            nc.vector.memset(wdup[:, 0:1], 1.0)
            nc.vector.tensor_tensor(
                out=wdup[:, 1:L], in0=xs[:, 1:L], in1=xs[:, 0:L - 1],
                op=mybir.AluOpType.not_equal,
            )

            # ---- device index math on sorted tokens -----------------------
            xi = pool.tile([P, L], I32)
            nc.vector.tensor_copy(out=xi[:], in_=xs[:])
            blk32 = pool.tile([P, L], I32)
            nc.vector.tensor_scalar(
                out=blk32[:], in0=xi[:], scalar1=2, scalar2=None,
                op0=mybir.AluOpType.arith_shift_right,
            )
            blk16 = pool.tile([P, L], I16)
            nc.vector.tensor_copy(out=blk16[:], in_=blk32[:])
            mm32 = pool.tile([P, L], I32)
            nc.vector.tensor_scalar(
                out=mm32[:], in0=xi[:], scalar1=3, scalar2=None,
                op0=mybir.AluOpType.bitwise_and,
            )
            mmf = pool.tile([P, L], F32)
            nc.vector.tensor_copy(out=mmf[:], in_=mm32[:])
            # lane masks m in 0..3, plain (for s) and dedup-weighted (for Wv)
            m4 = pool.tile([P, L * GB], F32)
            m4w = pool.tile([P, L * GB], F32)
            for m in range(GB):
                nc.vector.tensor_scalar(
                    out=m4[:, m * L:(m + 1) * L], in0=mmf[:],
                    scalar1=float(m), scalar2=None,
                    op0=mybir.AluOpType.is_equal,
                )
                nc.vector.tensor_tensor(
                    out=m4w[:, m * L:(m + 1) * L],
                    in0=m4[:, m * L:(m + 1) * L], in1=wdup[:],
                    op=mybir.AluOpType.mult,
                )

            # ---- wrap blk16 into dma_gather idx layout -------------------
            # idx k = j*128 + row -> wrapped slot (row%16, 8j + row//16),
            # replicated to all 8 partition groups. Partition fold via 8
            # partition-shift SBUF DMAs, then DVE stride-8 interleave, then
            # group replication (all contiguous descriptors).
            tmpw = pool.tile([16, L * 8], I16)   # [pp, 200q + j]
            for q in range(8):
                nc.sync.dma_start(
                    out=tmpw[:, L * q:L * (q + 1)],
                    in_=blk16[16 * q:16 * (q + 1), :],
                )
            idx0 = pool.tile([16, L * 8], I16)   # [pp, 8j + q]
            for q in range(8):
                nc.vector.tensor_copy(
                    out=idx0[:, q:L * 8:8], in_=tmpw[:, L * q:L * (q + 1)]
                )
            idx_sb = pool.tile([P, L * 8], I16)
            for g in range(8):
                nc.sync.dma_start(
                    out=idx_sb[16 * g:16 * (g + 1), :], in_=idx0[:]
                )

            # ---- stream E shard; s = (E @ wemb) / L -----------------------
            s_sb = pool.tile([P, TPP], F32)
            prod = pool.tile([P, EMB], F32)
            for g in range(NG):
                et = epool.tile([P, NG * EMB], F32, tag="et")
                nc.sync.dma_start(
                    out=et[:],
                    in_=e_sh[:].rearrange("(a b) e -> a b e", a=P)[
                        :, g * NG:(g + 1) * NG, :
                    ],
                )
                for u in range(NG):
                    t = g * NG + u
                    nc.vector.tensor_tensor(
                        out=prod[:], in0=et[:, u * EMB:(u + 1) * EMB],
                        in1=wb[:], op=mybir.AluOpType.mult,
                    )
                    nc.scalar.activation(
                        out=prod[:], in_=prod[:],
                        func=mybir.ActivationFunctionType.Copy,
                        scale=1.0, accum_out=s_sb[:, t:t + 1],
                    )
            # s_shard flat: value (p, t) -> local entry 49p + t
            nc.sync.dma_start(
                out=s_shard[0].rearrange("(a b) -> a b", a=P), in_=s_sb[:]
            )

            # ---- zero vt pad zone of s_full, then AllGather ----------------
            zpad = pool.tile([1, VTOT - VOCAB], F32)
            nc.vector.memset(zpad[:], 0.0)
            nc.sync.dma_start(out=s_full[0:1, VOCAB:VTOT], in_=zpad[:])
            nc.gpsimd.collective_compute(
                "AllGather", mybir.AluOpType.bypass,
                replica_groups=[list(range(N_CORES))],
                ins=[s_shard[0:1, 0:VSH]],
                outs=[s_full[0:1, 0:VOCAB].rearrange("o (a b) -> (o a) b", a=N_CORES)],
            )

            # ---- finish vt: interleave s lanes, write to DRAM -------------
            sf_sb = pool.tile([P, EPP], F32)
            nc.sync.dma_start(
                out=sf_sb[:], in_=s_full[0].rearrange("(p f) -> p f", p=P)
            )
            nc.vector.tensor_copy(
                out=vt_sb[:, 0:EPP * VT_ROW:VT_ROW], in_=sf_sb[:]
            )
            nc.sync.dma_start(out=vt[:], in_=vt_sb[:])

            # ---- gather values: 4 chunks of 6400 idx ----------------------
            accs = pool.tile([P, 2 * GB * NCHUNK], F32)  # 32 partial columns
            scrg = pool.tile([P, JC], F32)
            vt_rows = vt[:].rearrange("a b -> (a b)").rearrange(
                "(r e) -> r e", e=64
            )
            for t in range(NCHUNK):
                gout = gpool.tile([P, JC * 64], F32, tag="g")
                nc.gpsimd.dma_gather(
                    out_ap=gout[:].rearrange("p (j e) -> p j e", e=64),
                    in_ap=vt_rows,
                    idxs_ap=idx_sb[:, t * 8 * JC:(t + 1) * 8 * JC],
                    num_idxs=P * JC,
                    num_idxs_reg=P * JC,
                    elem_size=64,
                    elem_step=64,
                    single_packet=False,
                )
                g3 = gout[:].rearrange("p (j e) -> p j e", e=64)
                for m in range(GB):
                    col = t * 8 + m
                    nc.vector.tensor_tensor(
                        out=scrg[:], in0=g3[:, :, m * VT_ROW],
                        in1=m4[:, m * L + t * JC: m * L + (t + 1) * JC],
                        op=mybir.AluOpType.mult,
                    )
                    nc.vector.tensor_reduce(
                        out=accs[:, col:col + 1], in_=scrg[:],
                        axis=mybir.AxisListType.X, op=mybir.AluOpType.add,
                    )
                    nc.vector.tensor_tensor(
                        out=scrg[:], in0=g3[:, :, m * VT_ROW + 1],
                        in1=m4w[:, m * L + t * JC: m * L + (t + 1) * JC],
                        op=mybir.AluOpType.mult,
                    )
                    nc.vector.tensor_reduce(
                        out=accs[:, col + 4:col + 5], in_=scrg[:],
                        axis=mybir.AxisListType.X, op=mybir.AluOpType.add,
                    )

            # ---- logit + sigmoid ------------------------------------------
            pre = pool.tile([P, 1], F32)
            nc.vector.tensor_reduce(
                out=pre[:], in_=accs[:],
                axis=mybir.AxisListType.X, op=mybir.AluOpType.add,
            )
            y_sb = pool.tile([P, 1], F32)
            nc.scalar.activation(
                out=y_sb[:], in_=pre[:],
                func=mybir.ActivationFunctionType.Sigmoid,
                bias=bb[:, 0:1], scale=1.0,
            )
            nc.sync.dma_start(out=y_out[:], in_=y_sb[:])
    nc.compile()
    return nc


def prep_inputs(x, embedding_weight, W, b):
    """Host-side sharding/layout prep. Returns per-core input maps."""
    x = np.asarray(x)
    E = np.asarray(embedding_weight, dtype=np.float32)
    W = np.asarray(W, dtype=np.float32)
    b = np.asarray(b, dtype=np.float32)
    wemb = (W[0, :EMB] / L).reshape(1, EMB).astype(np.float32)
    wv_pad = np.zeros(VTOT, dtype=np.float32)
    wv_pad[:VOCAB] = W[0, EMB:]
    wv_all = wv_pad.reshape(P, EPP)
    xi = x.astype(np.int32)

    in_maps = []
    for c in range(N_CORES):
        e_c = np.zeros((VSHP, EMB), dtype=np.float32)
        e_c[:VSH] = E[c * VSH:(c + 1) * VSH]
        in_maps.append({
            "x_in": xi[c * P:(c + 1) * P],
            "e_sh": e_c,
            "wemb": wemb,
            "wv_all": wv_all,
            "bias_in": b.reshape(1, 1),
        })
    return in_maps


def kernel(**inputs):
    if "nc" not in _CACHE:
        _CACHE["nc"] = build_nc()
    nc = _CACHE["nc"]
    in_maps = prep_inputs(**inputs)
    from concourse.bass_utils import run_bass_kernel_spmd
    r = run_bass_kernel_spmd(nc, in_maps, list(range(N_CORES)))
    y = np.concatenate([r.results[c]["y_out"] for c in range(N_CORES)], axis=0)
    return y.astype(np.float32)



# revision 4
# speedup vs baseline: 2.6437x; 2.6437x over previous
"""Trainium2 Bass kernel for nn_LogisticRegression (multi-hot + mean-embedding
logistic regression over a 50k vocab).

Math: for each row i with tokens x[i, 0:200]:
    logit[i] = b + (1/200) * sum_j s[x_ij] + sum_{unique tokens t in row} Wv[t]
    y[i] = sigmoid(logit[i])
where s = E @ w_emb (one scalar per vocab entry), Wv = W[0, 300:].

Device strategy (8 NeuronCores, SPMD):
  - batch-shard rows: 128 rows per core (partition = row).
  - vocab-shard s: each core computes s for vocab ids [6272c, 6272(c+1))
    via PE matmuls on a host-transposed bf16 E shard (rhs = E^T tiles,
    lhsT = w broadcast to 128 columns, PSUM accumulation over 3 e-chunks).
  - a per-vocab fp8 table is built on device: entry e (=2 vocab ids) packs
    (ct[2e], s[2e], ct[2e+1], s[2e+1]) where ct = s/200 + Wv, scaled for
    fp8 range. Shards are AllGathered (12.5KB/core) then broadcast to all
    128 partitions (100KB/partition).
  - ONE gpsimd ap_gather (3200 idx/core, all 8 Q7 cores in parallel)
    looks up 4 fp8 lanes per token from the SBUF-resident table. This
    replaces per-token DMA-gather descriptor generation (the baseline
    bottleneck: ~8ns/token serialized on 2 Q7 cores).
  - dedup: rows are sorted (Max8+MatchReplace); first-occurrence tokens
    read the combined ct lane, duplicates read the s-only lane. The
    lane/dedup/scale selection is folded into one mask tensor, so a
    single fused multiply-reduce produces the logits. Exact for any
    duplicate count.
"""
import sys

sys.path.insert(0, "/opt/trn_rl_repo")

import numpy as np

import concourse.bass as bass
import concourse.bacc as bacc
import concourse.mybir as mybir
import concourse.tile as tile

N_CORES = 8
P = 128             # rows per core
L = 200             # tokens per row
VOCAB = 50000
EMB = 300
SHARD = 6272        # vocab ids per core (identity layout, zero-padded)
VTOT = SHARD * N_CORES   # 50176
NE = VTOT // 2      # ap_gather entries (2 vocab ids each) = 25088
TBYTES = NE * 4     # fp8 table bytes per partition = 100352
NCH = 13            # s chunks: 12x512 + 1x128 = 6272
SC_CT = 256.0       # fp8 scale for ct lane
SC_S = 16.0         # fp8 scale for s lane
F32 = mybir.dt.float32
BF16 = mybir.dt.bfloat16
F8 = mybir.dt.float8e4
I32 = mybir.dt.int32
I16 = mybir.dt.int16

_CACHE = {}


def build_nc():
    nc = bacc.Bacc("TRN2", target_bir_lowering=False, debug=True)
    x_in = nc.dram_tensor("x_in", [P, L], I32, kind="ExternalInput")
    eT0 = nc.dram_tensor("eT0", [128, SHARD], BF16, kind="ExternalInput")
    eT1 = nc.dram_tensor("eT1", [128, SHARD], BF16, kind="ExternalInput")
    eT2 = nc.dram_tensor("eT2", [44, SHARD], BF16, kind="ExternalInput")
    wbc_in = nc.dram_tensor("wbc_in", [EMB, 128], BF16, kind="ExternalInput")
    wv_in = nc.dram_tensor("wv_in", [P, 49], F32, kind="ExternalInput")
    bias_in = nc.dram_tensor("bias_in", [1, 1], F32, kind="ExternalInput")
    y_out = nc.dram_tensor("y_out", [P, 1], F32, kind="ExternalOutput")

    s_dram = nc.dram_tensor("s_dram", [1, SHARD], F32)
    ct_shard = nc.dram_tensor("ct_shard", [1, SHARD * 2], F8)
    ct_full = nc.dram_tensor("ct_full", [1, TBYTES], F8, addr_space="Shared")

    eT = [eT0, eT1, eT2]
    with tile.TileContext(nc) as tc:
        with (
            tc.tile_pool(name="sb", bufs=1) as pool,
            tc.tile_pool(name="et", bufs=2) as epool,
            tc.tile_pool(name="ps", bufs=4, space="PSUM") as psum_pool,
        ):
            # ---- prefault the ap_gather ucode library early ----------------
            dum_t = pool.tile([P, 64], F8)
            nc.vector.memset(dum_t[:], 0.0)
            dum_i = pool.tile([P, 1], I16)
            nc.vector.memset(dum_i[:], 0)
            dum_o = pool.tile([P, 64], F8)
            nc.gpsimd.ap_gather(
                out_ap=dum_o[:], in_ap=dum_t[:], idxs_ap=dum_i[:],
                channels=P, num_elems=16, d=4, num_idxs=16,
            )

            # ---- tokens: sort, dedup weights, lane masks, gather idx -------
            x_sb = pool.tile([P, L], I32)
            nc.sync.dma_start(out=x_sb[:], in_=x_in[:])
            xf = pool.tile([P, L], F32)
            nc.vector.tensor_copy(out=xf[:], in_=x_sb[:])
            xs = pool.tile([P, L], F32)
            work = pool.tile([P, L], F32)
            nc.vector.tensor_copy(out=work[:], in_=xf[:])
            for k in range(L // 8):
                nc.vector.max(out=xs[:, 8 * k:8 * k + 8], in_=work[:])
                nc.vector.match_replace(
                    out=work[:],
                    in_to_replace=xs[:, 8 * k:8 * k + 8],
                    in_values=work[:],
                    imm_value=-1.0,
                )
            w1 = pool.tile([P, L], F32)
            nc.vector.memset(w1[:, 0:1], 1.0)
            nc.vector.tensor_tensor(
                out=w1[:, 1:L], in0=xs[:, 1:L], in1=xs[:, 0:L - 1],
                op=mybir.AluOpType.not_equal,
            )
            xi = pool.tile([P, L], I32)
            nc.vector.tensor_copy(out=xi[:], in_=xs[:])
            band = pool.tile([P, L], I32)
            nc.vector.tensor_scalar(
                out=band[:], in0=xi[:], scalar1=1, scalar2=None,
                op0=mybir.AluOpType.bitwise_and,
            )
            bf = pool.tile([P, L], F32)
            nc.vector.tensor_copy(out=bf[:], in_=band[:])
            ei32 = pool.tile([P, L], I32)
            nc.vector.tensor_scalar(
                out=ei32[:], in0=xi[:], scalar1=1, scalar2=None,
                op0=mybir.AluOpType.arith_shift_right,
            )
            idx16 = pool.tile([P, L], I16)
            nc.vector.tensor_copy(out=idx16[:], in_=ei32[:])

            # masks: lane (2q = ct, 2q+1 = s) x dedup weight x fp8 unscale
            t0 = pool.tile([P, L], F32)   # w1 / SC_CT
            nc.vector.tensor_scalar(
                out=t0[:], in0=w1[:], scalar1=1.0 / SC_CT, scalar2=None,
                op0=mybir.AluOpType.mult,
            )
            t1 = pool.tile([P, L], F32)   # (1 - w1) / (SC_S * L)
            nc.vector.tensor_scalar(
                out=t1[:], in0=w1[:], scalar1=-1.0 / (SC_S * L),
                scalar2=1.0 / (SC_S * L),
                op0=mybir.AluOpType.mult, op1=mybir.AluOpType.add,
            )
            bn = pool.tile([P, L], F32)   # 1 - bf
            nc.vector.tensor_scalar(
                out=bn[:], in0=bf[:], scalar1=-1.0, scalar2=1.0,
                op0=mybir.AluOpType.mult, op1=mybir.AluOpType.add,
            )
            mask4 = pool.tile([P, L * 4], F32)
            m4v = mask4[:].rearrange("p (j m) -> p j m", m=4)
            nc.vector.tensor_tensor(
                out=m4v[:, :, 0], in0=t0[:], in1=bn[:],
                op=mybir.AluOpType.mult,
            )
            nc.vector.tensor_tensor(
                out=m4v[:, :, 1], in0=t1[:], in1=bn[:],
                op=mybir.AluOpType.mult,
            )
            nc.vector.tensor_tensor(
                out=m4v[:, :, 2], in0=t0[:], in1=bf[:],
                op=mybir.AluOpType.mult,
            )
            nc.vector.tensor_tensor(
                out=m4v[:, :, 3], in0=t1[:], in1=bf[:],
                op=mybir.AluOpType.mult,
            )

            # ---- s shard via PE: s = E_shard @ w_emb -----------------------
            wbc = []
            for e, rows in enumerate((128, 128, 44)):
                wt = pool.tile([rows, 128], BF16, tag=f"wbc{e}")
                nc.sync.dma_start(
                    out=wt[:], in_=wbc_in[128 * e:128 * e + rows, :]
                )
                wbc.append(wt)
            s1 = pool.tile([1, SHARD], F32)
            # groups of 3 chunks (1536 cols); last group = 1 chunk of 128
            groups = [(g * 1536, (512, 512, 512)) for g in range(4)]
            groups.append((6144, (128,)))
            for base, chunks in groups:
                width = sum(chunks)
                ets = []
                for e, rows in enumerate((128, 128, 44)):
                    et = epool.tile([rows, width], BF16, tag=f"et{e}")
                    nc.sync.dma_start(
                        out=et[:], in_=eT[e][:, base:base + width]
                    )
                    ets.append(et)
                off = 0
                for w in chunks:
                    ps = psum_pool.tile([128, w], F32)
                    for e in range(3):
                        nc.tensor.matmul(
                            ps[:],
                            wbc[e][:],
                            ets[e][:, off:off + w],
                            start=(e == 0), stop=(e == 2),
                        )
                    nc.scalar.copy(
                        out=s1[0:1, base + off:base + off + w], in_=ps[0:1, :]
                    )
                    off += w

            # ---- build fp8 table shard, AllGather, broadcast ---------------
            nc.sync.dma_start(out=s_dram[0:1, :], in_=s1[:])
            s128 = pool.tile([P, 49], F32)
            nc.sync.dma_start(
                out=s128[:], in_=s_dram[0].rearrange("(p g) -> p g", p=P)
            )
            wv128 = pool.tile([P, 49], F32)
            nc.sync.dma_start(out=wv128[:], in_=wv_in[:])
            ctsh = pool.tile([P, 98], F8)
            cv = ctsh[:].rearrange("p (g t) -> p g t", t=2)
            nc.vector.scalar_tensor_tensor(
                out=cv[:, :, 0], in0=s128[:], scalar=SC_CT / L, in1=wv128[:],
                op0=mybir.AluOpType.mult, op1=mybir.AluOpType.add,
            )
            nc.vector.tensor_scalar(
                out=cv[:, :, 1], in0=s128[:], scalar1=SC_S / L, scalar2=None,
                op0=mybir.AluOpType.mult,
            )
            nc.sync.dma_start(
                out=ct_shard[0:1, :].rearrange("o (p b) -> (o p) b", p=P),
                in_=ctsh[:],
            )
            nc.gpsimd.collective_compute(
                "AllGather", mybir.AluOpType.bypass,
                replica_groups=[list(range(N_CORES))],
                ins=[ct_shard[0:1, :]],
                outs=[ct_full[0:1, :].rearrange("o (a b) -> (o a) b", a=N_CORES)],
            )
            table = pool.tile([P, TBYTES], F8)
            nc.sync.dma_start(
                out=table[:], in_=ct_full[0:1, :].to_broadcast([P, TBYTES])
            )

            # ---- the gather + fused masked reduce --------------------------
            gout = pool.tile([P, L * 16 * 4], F8)
            nc.gpsimd.ap_gather(
                out_ap=gout[:], in_ap=table[:], idxs_ap=idx16[:],
                channels=P, num_elems=NE, d=4, num_idxs=L * 16,
            )
            ge = pool.tile([P, L * 4], F8)
            gv = gout[:].rearrange(
                "(g w) (j w2 m) -> g w j w2 m", w=16, w2=16, m=4
            )
            gev = ge[:].rearrange("(g w) (j m) -> g w j m", w=16, m=4)
            for w in range(16):
                nc.sync.dma_start(out=gev[:, w], in_=gv[:, w, :, w, :])
            gef = pool.tile([P, L * 4], F32)
            nc.vector.tensor_copy(out=gef[:], in_=ge[:])
            junk = pool.tile([P, L * 4], F32)
            acc = pool.tile([P, 1], F32)
            nc.vector.tensor_tensor_reduce(
                out=junk[:], in0=gef[:], in1=mask4[:], scale=1.0, scalar=0.0,
                op0=mybir.AluOpType.mult, op1=mybir.AluOpType.add,
                accum_out=acc[:],
            )

            # ---- sigmoid(acc + b) ------------------------------------------
            bb = pool.tile([P, 1], F32)
            nc.sync.dma_start(out=bb[:], in_=bias_in[0:1, :].to_broadcast([P, 1]))
            y_sb = pool.tile([P, 1], F32)
            nc.scalar.activation(
                out=y_sb[:], in_=acc[:],
                func=mybir.ActivationFunctionType.Sigmoid,
                bias=bb[:, 0:1], scale=1.0,
            )
            nc.sync.dma_start(out=y_out[:], in_=y_sb[:])
    nc.compile()
    return nc


def prep_inputs(x, embedding_weight, W, b):
    """Host-side sharding/layout prep (data-independent reformatting)."""
    import ml_dtypes
    x = np.asarray(x)
    E = np.asarray(embedding_weight, dtype=np.float32)
    W = np.asarray(W, dtype=np.float32)
    b = np.asarray(b, dtype=np.float32)
    wemb = W[0, :EMB]
    Wv = W[0, EMB:]

    wbc = np.repeat(wemb[:, None], 128, axis=1).astype(ml_dtypes.bfloat16)
    wv_pad = np.zeros(VTOT, dtype=np.float32)
    wv_pad[:VOCAB] = Wv * SC_CT
    xi = x.astype(np.int32)

    in_maps = []
    for c in range(N_CORES):
        lo = SHARD * c
        hi = min(SHARD * (c + 1), VOCAB)
        esh = np.zeros((SHARD, EMB), dtype=np.float32)
        esh[:hi - lo] = E[lo:hi]
        eshT = np.ascontiguousarray(esh.T).astype(ml_dtypes.bfloat16)
        in_maps.append({
            "x_in": xi[c * P:(c + 1) * P],
            "eT0": eshT[0:128],
            "eT1": eshT[128:256],
            "eT2": eshT[256:300],
            "wbc_in": wbc,
            "wv_in": wv_pad[lo:lo + SHARD].reshape(P, 49),
            "bias_in": b.reshape(1, 1),
        })
    return in_maps


def kernel(**inputs):
    if "nc" not in _CACHE:
        _CACHE["nc"] = build_nc()
    nc = _CACHE["nc"]
    in_maps = prep_inputs(**inputs)
    from concourse.bass_utils import run_bass_kernel_spmd
    r = run_bass_kernel_spmd(nc, in_maps, list(range(N_CORES)))
    y = np.concatenate([r.results[c]["y_out"] for c in range(N_CORES)], axis=0)
    return y.astype(np.float32)
